# revision 19
# baseline (speedup 1.0000x reference)
"""Trainium2 Bass kernel for nn_CausalSelfAttention (B=1, S=2048, D=1024, H=16).

Tensor-parallel over heads across 8 NeuronCores: core c computes heads
(2c, 2c+1) end-to-end and the row-parallel slice of the output
projection; the host sums the 8 fp16 partial outputs in fp32.

v3 design (vs v2):
  - all-fp16 PE datapath; the fp32r broadcast matmuls are gone (rms/denom
    broadcasts run as fp16 matmuls at full rate).
  - rsqrt for the rms norms runs as ln->exp on ACT; every ACT function
    used (Ln/Exp/Copy) lives in one activation table, so the 1283ns ACT
    table reloads disappear.
  - v projection is weight-stationary (full-rate 512-col streams) followed
    by PE transposes into [kpos, m]; lam*v1 (preloaded in v_ext) is folded
    by DVE adds during the transpose evac.
  - the causal diag mask is a 0/1 multiply on the Pool engine after exp;
    scores matmuls carry no mask matmuls.
  - jloop is software-pipelined: scores(j+1) issue before PV(j), with
    half-1 QKV projection / v path / output-projection tails injected
    into the stagger so the PE never drains.
  - PSUM tag plan, exactly 8 banks: "qk" 2x[128,1024]f32 (8KB),
    "v" 2x2KB small slots, "n" 1x[65,1024]f32 (4KB).
"""

import os
import sys

import numpy as np

try:
    import concourse.bass as bass  # noqa: F401
except Exception:  # pragma: no cover
    for _p in ("/opt/trn_rl_repo", "/root/.axon_site/_ro/trn_rl_repo"):
        if os.path.isdir(_p) and _p not in sys.path:
            sys.path.insert(0, _p)

import concourse.bacc as bacc
import concourse.bass as bass
import concourse.mybir as mybir
import concourse.tile as tile
from concourse import bass_utils

S = 2048
D = 1024
NH = 16
HD = 64
NCORES = 8
NCH = D // 128           # 8 contraction chunks
NKC = S // 128           # 16 k chunks of 128
NST = S // 512           # 4 q strips of 512
HALF = S // 2

F32 = mybir.dt.float32
F16 = mybir.dt.float16
AF = mybir.ActivationFunctionType

EPS = float(np.finfo(np.float32).eps)
EXPB = -5.545      # exp bias: keeps fp16 exp outputs < 65504; cancels in softmax


def _emit(tc, io):
    nc = tc.nc

    consts = tc.alloc_tile_pool(name="consts", bufs=1)
    wpool = tc.alloc_tile_pool(name="wpool", bufs=1)
    persist = tc.alloc_tile_pool(name="persist", bufs=1)
    work = tc.alloc_tile_pool(name="work", bufs=2)
    xtp = tc.alloc_tile_pool(name="xt", bufs=1)
    late = tc.alloc_tile_pool(name="late", bufs=1)
    pp = tc.alloc_tile_pool(name="pp", bufs=1, space="PSUM")

    # ---- DMAs: wq + first xT chunks first so QKV starts ASAP ----------
    xt_sb = xtp.tile([128, NCH, S], F16)
    wqkv = wpool.tile([128, 3, NCH, 128], F16)
    w_sb = {nm: wqkv[:, i, :, :] for i, nm in enumerate(("wq", "wk", "wv"))}

    nc.sync.dma_start(out=wqkv[:, 0, :, :], in_=io["wqkv"].ap()[:, 0, :, :])
    nc.sync.dma_start(out=xt_sb[:, 0:2, 0:HALF], in_=io["xT"].ap()[:, 0:2, 0:HALF])
    nc.sync.dma_start(out=wqkv[:, 1:3, :, :], in_=io["wqkv"].ap()[:, 1:3, :, :])
    for c0 in (2, 4, 6):
        nc.sync.dma_start(out=xt_sb[:, c0:c0 + 2, 0:HALF],
                          in_=io["xT"].ap()[:, c0:c0 + 2, 0:HALF])
    nc.sync.dma_start(out=xt_sb[:, 0:4, HALF:S], in_=io["xT"].ap()[:, 0:4, HALF:S])
    nc.sync.dma_start(out=xt_sb[:, 4:8, HALF:S], in_=io["xT"].ap()[:, 4:8, HALF:S])

    # cold consts on the gpsimd queue
    ident = consts.tile([128, 128], F16)
    nc.gpsimd.dma_start(out=ident, in_=io["ident"].ap())
    tri2 = consts.tile([128, 2, 128], F16)     # 0/1 causal keep-mask, both heads
    nc.gpsimd.dma_start(out=tri2, in_=io["tri2"].ap())
    ind8 = consts.tile([128, 8], F16)
    nc.gpsimd.dma_start(out=ind8, in_=io["ind8"].ap())
    indT4 = consts.tile([4, 2, 128], F16)      # [:,0,:] q head-map, [:,1,:] k
    nc.gpsimd.dma_start(out=indT4, in_=io["indT4"].ap())
    lnsb = consts.tile([4, 2], F32)            # [:,0] ln scale, [:,1] ln bias
    nc.gpsimd.dma_start(out=lnsb, in_=io["lnsb"].ap())
    trig = consts.tile([128, 2, S], F16)       # [cos | sin] packed
    nc.gpsimd.dma_start(out=trig, in_=io["trig"].ap())
    cosT = trig[:, 0, :]
    sinTs = trig[:, 1, :]
    wo_sb = wpool.tile([128, D], F16)
    nc.gpsimd.dma_start(out=wo_sb, in_=io["wo"].ap())

    ebias = consts.tile([128, 1], F32)
    nc.vector.memset(ebias, EXPB)
    zero4 = consts.tile([4, 1], F32)
    nc.vector.memset(zero4, 0.0)
    ind64 = consts.tile([1, 64], F16)
    nc.vector.memset(ind64, 1.0)

    # ---- persistent activations --------------------------------------
    q_fin = persist.tile([128, S], F16)
    k_fin = persist.tile([128, S], F16)
    # v_ext [k, chunk, (h,65)]: preloaded with lam*v1 (+ones at 64/129);
    # the PE-transposed device v-projection is ADDED in by DVE.
    v_ext = persist.tile([128, NKC, 130], F16)
    nc.gpsimd.dma_start(out=v_ext, in_=io["v1e"].ap())
    y2T = persist.tile([128, S], F16)

    SWAPS = ((0, 32), (32, 0), (64, 96), (96, 64))

    # ================= half-0 prep =====================================
    # PE: psq0(16) psk0(16) psv0a(8) psv0b(8) pst0(8tr) psn0 pse{q,k}0
    ps_q0 = {}
    for l0 in (0, 512):
        ps_q0[l0] = pp.tile([128, 512], F32, tag="qk", bufs=2, name=f"psq0_{l0}")
        for c in range(NCH):
            nc.tensor.matmul(ps_q0[l0], w_sb["wq"][:, c, :],
                             xt_sb[:, c, l0:l0 + 512],
                             start=(c == 0), stop=(c == NCH - 1))
    # DVE followers for q0
    qraw0 = work.tile([128, HALF], F16, tag="rawq", bufs=2, name="qraw0")
    sq_q0 = work.tile([128, HALF], F16, tag="sqq", bufs=2, name="sqq0")
    for l0 in (0, 512):
        nc.vector.tensor_copy(out=qraw0[:, l0:l0 + 512], in_=ps_q0[l0])
        nc.vector.tensor_mul(out=sq_q0[:, l0:l0 + 512],
                             in0=qraw0[:, l0:l0 + 512], in1=qraw0[:, l0:l0 + 512])
        nc.vector.tensor_mul(out=q_fin[:, l0:l0 + 512],
                             in0=qraw0[:, l0:l0 + 512], in1=cosT[:, l0:l0 + 512])
    swq0 = work.tile([128, HALF], F16, tag="swq", bufs=2, name="swq0")
    for d0, sf in SWAPS:
        nc.gpsimd.dma_start(out=swq0[d0:d0 + 32, :], in_=qraw0[sf:sf + 32, :])

    ps_k0 = {}
    for l0 in (0, 512):
        ps_k0[l0] = pp.tile([128, 512], F32, tag="qk", bufs=2, name=f"psk0_{l0}")
        for c in range(NCH):
            nc.tensor.matmul(ps_k0[l0], w_sb["wk"][:, c, :],
                             xt_sb[:, c, l0:l0 + 512],
                             start=(c == 0), stop=(c == NCH - 1))
    kraw0 = work.tile([128, HALF], F16, tag="rawk", bufs=2, name="kraw0")
    sq_k0 = work.tile([128, HALF], F16, tag="sqk", bufs=2, name="sqk0")
    for l0 in (0, 512):
        nc.vector.tensor_copy(out=kraw0[:, l0:l0 + 512], in_=ps_k0[l0])
        nc.vector.tensor_mul(out=sq_k0[:, l0:l0 + 512],
                             in0=kraw0[:, l0:l0 + 512], in1=kraw0[:, l0:l0 + 512])
        nc.vector.tensor_mul(out=k_fin[:, l0:l0 + 512],
                             in0=kraw0[:, l0:l0 + 512], in1=cosT[:, l0:l0 + 512])
    swk0 = work.tile([128, HALF], F16, tag="swk", bufs=2, name="swk0")
    for d0, sf in SWAPS:
        nc.gpsimd.dma_start(out=swk0[d0:d0 + 32, :], in_=kraw0[sf:sf + 32, :])

    # v projection (weight-stationary) + transposes into one packed tile
    vmk0 = work.tile([128, HALF], F16, tag="vmk", bufs=2, name="vmk0")
    for g in (0, 1):
        ps_v = pp.tile([128, 512], F32, tag="v", bufs=2, name=f"psv0{g}")
        for c in range(NCH):
            nc.tensor.matmul(ps_v, w_sb["wv"][:, c, :],
                             xt_sb[:, c, 512 * g:512 * g + 512],
                             start=(c == 0), stop=(c == NCH - 1))
        nc.vector.tensor_copy(out=vmk0[:, 512 * g:512 * g + 512], in_=ps_v)

    # rope mixing h0 (DVE): q_fin += swap*sin ; same for k
    for l0 in (0, 512):
        nc.vector.tensor_mul(out=swq0[:, l0:l0 + 512], in0=swq0[:, l0:l0 + 512],
                             in1=sinTs[:, l0:l0 + 512])
        nc.vector.tensor_add(out=q_fin[:, l0:l0 + 512], in0=q_fin[:, l0:l0 + 512],
                             in1=swq0[:, l0:l0 + 512])
    for l0 in (0, 512):
        nc.vector.tensor_mul(out=swk0[:, l0:l0 + 512], in0=swk0[:, l0:l0 + 512],
                             in1=sinTs[:, l0:l0 + 512])
        nc.vector.tensor_add(out=k_fin[:, l0:l0 + 512], in0=k_fin[:, l0:l0 + 512],
                             in1=swk0[:, l0:l0 + 512])

    # norm sums + ln/exp rsqrt + fp16 broadcast, half 0, per l0 block
    lnv0 = work.tile([4, HALF], F32, tag="lnv", bufs=2, name="lnv0")
    inv40 = work.tile([4, HALF], F16, tag="inv4", bufs=2, name="inv40")
    ps_eq0 = pp.tile([128, HALF], F32, tag="qk", bufs=2, name="pseq0")
    ps_ek0 = pp.tile([128, HALF], F32, tag="qk", bufs=2, name="psek0")

    def psn_block(H, sq_q, sq_k, lnv, inv4, l0):
        ps_n = pp.tile([4, 512], F32, tag="v", bufs=2, name=f"psn{H}_{l0}")
        nc.tensor.matmul(ps_n, ind8[:, 0:4], sq_q[:, l0:l0 + 512],
                         start=True, stop=False)
        nc.tensor.matmul(ps_n, ind8[:, 4:8], sq_k[:, l0:l0 + 512],
                         start=False, stop=True)
        nc.scalar.activation(out=lnv[:, l0:l0 + 512], in_=ps_n, func=AF.Ln,
                             bias=lnsb[:, 1:2], scale=lnsb[:, 0:1])
        nc.scalar.activation(out=inv4[:, l0:l0 + 512], in_=lnv[:, l0:l0 + 512],
                             func=AF.Exp, bias=zero4, scale=-0.5)

    for l0 in (0, 512):
        psn_block(0, sq_q0, sq_k0, lnv0, inv40, l0)

    # v transposes after the psn tiles (tag-v rotation order matters)
    pst0 = pp.tile([128, 8, 128], F16, tag="v", bufs=2, name="pst0")
    for kci in range(8):
        nc.tensor.transpose(pst0[:, kci, :], vmk0[:, 128 * kci:128 * kci + 128],
                            ident)

    for l0 in (0, 512):
        nc.tensor.matmul(ps_eq0[:, l0:l0 + 512], indT4[:, 0, :],
                         inv40[:, l0:l0 + 512], start=True, stop=True)
        nc.tensor.matmul(ps_ek0[:, l0:l0 + 512], indT4[:, 1, :],
                         inv40[:, l0:l0 + 512], start=True, stop=True)

    def vext_add(kc, pst, kci):
        dst = v_ext[:, kc, :].rearrange("p (h n) -> p h n", h=2)[:, :, 0:64]
        src = pst[:, kci, :].rearrange("p (h n) -> p h n", h=2)
        nc.vector.tensor_add(out=dst, in0=src, in1=dst)

    # DVE: scale muls interleaved with v_ext adds (pv(0,0) needs kc0 early)
    nc.vector.tensor_mul(out=q_fin[:, 0:512], in0=q_fin[:, 0:512],
                         in1=ps_eq0[:, 0:512])
    nc.vector.tensor_mul(out=k_fin[:, 0:512], in0=k_fin[:, 0:512],
                         in1=ps_ek0[:, 0:512])
    for kci in range(4):
        vext_add(kci, pst0, kci)
    nc.vector.tensor_mul(out=q_fin[:, 512:1024], in0=q_fin[:, 512:1024],
                         in1=ps_eq0[:, 512:1024])
    nc.vector.tensor_mul(out=k_fin[:, 512:1024], in0=k_fin[:, 512:1024],
                         in1=ps_ek0[:, 512:1024])
    for kci in range(4, 8):
        vext_add(kci, pst0, kci)

    # ================= half-1 prep units (injected into jloop) =========
    h1 = {}
    h1["qraw"] = work.tile([128, HALF], F16, tag="rawq", bufs=2, name="qraw1")
    h1["sqq"] = work.tile([128, HALF], F16, tag="sqq", bufs=2, name="sqq1")
    h1["swq"] = work.tile([128, HALF], F16, tag="swq", bufs=2, name="swq1")
    h1["kraw"] = work.tile([128, HALF], F16, tag="rawk", bufs=2, name="kraw1")
    h1["sqk"] = work.tile([128, HALF], F16, tag="sqk", bufs=2, name="sqk1")
    h1["swk"] = work.tile([128, HALF], F16, tag="swk", bufs=2, name="swk1")
    h1["vmk"] = work.tile([128, HALF], F16, tag="vmk", bufs=2, name="vmk1")
    h1["lnv"] = work.tile([4, HALF], F32, tag="lnv", bufs=2, name="lnv1")
    h1["inv4"] = work.tile([4, HALF], F16, tag="inv4", bufs=2, name="inv41")
    h1["ps"] = {}

    def proj1(which, part):
        """part 0..3: quarter of the half-1 q/k projection, tag-v psum."""
        l0 = 0 if part < 2 else 512
        key = f"{which}{l0}"
        if part % 2 == 0:
            h1["ps"][key] = pp.tile([128, 512], F32, tag="v", bufs=2,
                                    name=f"ps{which}1_{l0}")
        ps = h1["ps"][key]
        cs = range(0, 4) if part % 2 == 0 else range(4, 8)
        for c in cs:
            nc.tensor.matmul(ps, w_sb[f"w{which}"][:, c, :],
                             xt_sb[:, c, HALF + l0:HALF + l0 + 512],
                             start=(c == 0), stop=(c == NCH - 1))
        if part % 2 == 1:
            raw = h1["qraw" if which == "q" else "kraw"]
            sq = h1["sqq" if which == "q" else "sqk"]
            fin = q_fin if which == "q" else k_fin
            s = HALF + l0
            nc.vector.tensor_copy(out=raw[:, l0:l0 + 512], in_=ps)
            nc.vector.tensor_mul(out=sq[:, l0:l0 + 512],
                                 in0=raw[:, l0:l0 + 512], in1=raw[:, l0:l0 + 512])
            nc.vector.tensor_mul(out=fin[:, s:s + 512],
                                 in0=raw[:, l0:l0 + 512], in1=cosT[:, s:s + 512])
        if part == 3:
            raw = h1["qraw" if which == "q" else "kraw"]
            sw = h1["swq" if which == "q" else "swk"]
            fin = q_fin if which == "q" else k_fin
            for d0, sf in SWAPS:
                nc.gpsimd.dma_start(out=sw[d0:d0 + 32, :], in_=raw[sf:sf + 32, :])
            for l0b in (0, 512):
                s = HALF + l0b
                nc.vector.tensor_mul(out=sw[:, l0b:l0b + 512],
                                     in0=sw[:, l0b:l0b + 512], in1=sinTs[:, s:s + 512])
                nc.vector.tensor_add(out=fin[:, s:s + 512], in0=fin[:, s:s + 512],
                                     in1=sw[:, l0b:l0b + 512])

    def v1_proj(g):
        ps_v = pp.tile([128, 512], F32, tag="v", bufs=2, name=f"psv1{g}")
        for c in range(NCH):
            nc.tensor.matmul(ps_v, w_sb["wv"][:, c, :],
                             xt_sb[:, c, HALF + 512 * g:HALF + 512 * g + 512],
                             start=(c == 0), stop=(c == NCH - 1))
        nc.vector.tensor_copy(out=h1["vmk"][:, 512 * g:512 * g + 512], in_=ps_v)

    def psn1():
        for l0 in (0, 512):
            psn_block(1, h1["sqq"], h1["sqk"], h1["lnv"], h1["inv4"], l0)

    def bc1():
        ps_eq1 = pp.tile([128, HALF], F32, tag="qk", bufs=2, name="pseq1")
        ps_ek1 = pp.tile([128, HALF], F32, tag="qk", bufs=2, name="psek1")
        for l0 in (0, 512):
            nc.tensor.matmul(ps_eq1[:, l0:l0 + 512], indT4[:, 0, :],
                             h1["inv4"][:, l0:l0 + 512], start=True, stop=True)
            nc.tensor.matmul(ps_ek1[:, l0:l0 + 512], indT4[:, 1, :],
                             h1["inv4"][:, l0:l0 + 512], start=True, stop=True)
        for l0 in (0, 512):
            s = HALF + l0
            nc.vector.tensor_mul(out=q_fin[:, s:s + 512], in0=q_fin[:, s:s + 512],
                                 in1=ps_eq1[:, l0:l0 + 512])
            nc.vector.tensor_mul(out=k_fin[:, s:s + 512], in0=k_fin[:, s:s + 512],
                                 in1=ps_ek1[:, l0:l0 + 512])

    def vt1(phase):
        if phase == 0:
            h1["pst"] = pp.tile([128, 8, 128], F16, tag="v", bufs=2, name="pst1")
            for kci in range(8):
                nc.tensor.transpose(h1["pst"][:, kci, :],
                                    h1["vmk"][:, 128 * kci:128 * kci + 128], ident)
            for kci in range(4):
                vext_add(8 + kci, h1["pst"], kci)
        else:
            for kci in range(4, 8):
                vext_add(8 + kci, h1["pst"], kci)

    # ================= jloop ===========================================
    yraw = [None] * NST
    dinvs = [None] * NST
    yts_t = [None] * NST
    osb_t = [None] * NST
    ex_sl = {}
    sc_meta = {}

    def sc_exp(t, j):
        q0 = 512 * t
        off = max(0, 128 * j - q0)
        diag = off > 0 or 128 * j == q0
        if j == 0:
            yts_t[t] = pp.tile([65, 1024], F32, tag="n", bufs=1, name=f"yt{t}")
        sc = pp.tile([128, 1024], F32, tag="qk", bufs=2, name=f"sc{t}_{j}")
        for h in range(2):
            nc.tensor.matmul(
                sc[:, 512 * h + off:512 * h + 512],
                k_fin[64 * h:64 * h + 64, 128 * j:128 * j + 128],
                q_fin[64 * h:64 * h + 64, q0 + off:q0 + 512],
                start=True, stop=True)
        ex = late.tile([128, 1024], F16, tag="ex", bufs=4, name=f"ex{t}_{j}")
        sc_v = sc[:, :].rearrange("p (h n) -> p h n", h=2)[:, :, off:512]
        ex_v = ex[:, :].rearrange("p (h n) -> p h n", h=2)[:, :, off:512]
        nc.scalar.activation(out=ex_v, in_=sc_v, func=AF.Exp, bias=ebias)
        if diag:
            # zero the not-yet-valid triangle on Pool (0/1 multiply)
            mv = ex[:, :].rearrange("p (h n) -> p h n", h=2)[:, :, off:off + 128]
            nc.gpsimd.tensor_mul(out=mv, in0=mv, in1=tri2)
        ex_sl[(t, j)] = ex
        sc_meta[(t, j)] = (off, diag)

    def strip_done(t):
        """yraw evac + denominator reciprocal chain (DVE)."""
        den = late.tile([1, 1024], F32, tag="den", bufs=4, name=f"den{t}")
        nc.vector.tensor_copy(out=den, in_=yts_t[t][64:65, :])
        yr = late.tile([65, 1024], F16, tag="yr", bufs=4, name=f"yraw{t}")
        nc.vector.tensor_copy(out=yr, in_=yts_t[t])
        yraw[t] = yr
        scr = late.tile([1, 1024], F32, tag="scr", bufs=4, name=f"scr{t}")
        nc.vector.reciprocal_approx_fast(out=scr, in_=den)
        dinv = late.tile([1, 1024], F16, tag="dinv", bufs=4, name=f"dinv{t}")
        nc.vector.tensor_copy(out=dinv, in_=scr)
        dinvs[t] = dinv

    def pv(t, j):
        q0 = 512 * t
        njs = 4 * t + 4
        off, diag = sc_meta[(t, j)]
        ex = ex_sl[(t, j)]
        yts = yts_t[t]
        for h in range(2):
            vst = v_ext[:, j, 65 * h:65 * h + 65]
            yto = 512 * h
            if diag and off + 128 < 512:
                nc.tensor.matmul(yts[:, yto + off:yto + off + 128], vst,
                                 ex[:, 512 * h + off:512 * h + off + 128],
                                 start=(j == 0), stop=True)
                nc.tensor.matmul(yts[:, yto + off + 128:yto + 512], vst,
                                 ex[:, 512 * h + off + 128:512 * h + 512],
                                 start=False, stop=(j == njs - 1))
            elif diag:
                nc.tensor.matmul(yts[:, yto + off:yto + 512], vst,
                                 ex[:, 512 * h + off:512 * h + 512],
                                 start=(j == 0), stop=True)
            else:
                nc.tensor.matmul(yts[:, yto:yto + 512], vst,
                                 ex[:, 512 * h:512 * h + 512],
                                 start=(j == 0), stop=False)
        if j == njs - 1:
            strip_done(t)

    def tail_ibcast(t):
        q0 = 512 * t
        for h in range(2):
            ps_i = pp.tile([64, 512], F32, tag="v", bufs=2, name=f"psi{t}_{h}")
            nc.tensor.matmul(ps_i, ind64, dinvs[t][0:1, 512 * h:512 * h + 512],
                             start=True, stop=True)
            nc.vector.tensor_mul(out=y2T[64 * h:64 * h + 64, q0:q0 + 512],
                                 in0=yraw[t][0:64, 512 * h:512 * h + 512],
                                 in1=ps_i)

    def tail_wout(t, oc0):
        q0 = 512 * t
        if osb_t[t] is None:
            osb_t[t] = late.tile([128, 8, 512], F16, tag="osb", bufs=2,
                                 name=f"osb{t}")
        osb = osb_t[t]
        for oc in (oc0, oc0 + 1):
            ps_o = pp.tile([128, 512], F32, tag="v", bufs=2, name=f"po{t}_{oc}")
            nc.tensor.matmul(ps_o, wo_sb[:, 128 * oc:128 * oc + 128],
                             y2T[:, q0:q0 + 512], start=True, stop=True)
            if oc in ((1, 3, 5, 7) if t == 3 else (3, 7)):
                nc.scalar.copy(out=osb[:, oc, :], in_=ps_o)
            else:
                nc.vector.tensor_copy(out=osb[:, oc, :], in_=ps_o)
        if oc0 == 2:
            nc.gpsimd.dma_start(out=io["outp"].ap()[:, 0:4, q0:q0 + 512],
                                in_=osb[:, 0:4, :])
        elif oc0 == 6:
            nc.gpsimd.dma_start(out=io["outp"].ap()[:, 4:8, q0:q0 + 512],
                                in_=osb[:, 4:8, :])

    INJ = {
        0: lambda: proj1("q", 0),
        1: lambda: proj1("q", 1),
        2: lambda: proj1("q", 2),
        3: lambda: proj1("q", 3),
        4: lambda: proj1("k", 0),
        5: lambda: proj1("k", 1),
        6: lambda: proj1("k", 2),
        7: lambda: proj1("k", 3),
        8: psn1,
        9: bc1,
        10: lambda: v1_proj(0),
        11: lambda: v1_proj(1),
        12: lambda: vt1(0),
        13: lambda: vt1(1),
        16: lambda: tail_ibcast(0),
        17: lambda: tail_wout(0, 0),
        18: lambda: tail_wout(0, 2),
        19: lambda: tail_wout(0, 4),
        20: lambda: tail_wout(0, 6),
        21: lambda: tail_ibcast(1),
        22: lambda: tail_wout(1, 0),
        23: lambda: tail_wout(1, 2),
        24: lambda: tail_wout(1, 4),
        25: lambda: tail_wout(1, 6),
        28: lambda: tail_ibcast(2),
        29: lambda: tail_wout(2, 0),
        30: lambda: tail_wout(2, 2),
        31: lambda: tail_wout(2, 4),
        32: lambda: tail_wout(2, 6),
    }

    SLOTS = [(t, j) for t in range(NST) for j in range(4 * t + 4)]
    prev = None
    for i, (t, j) in enumerate(SLOTS):
        sc_exp(t, j)
        fn = INJ.get(i)
        if fn is not None:
            fn()
        if prev is not None:
            pv(*prev)
        prev = (t, j)
    pv(*prev)

    # final tail (strip 3)
    tail_ibcast(3)
    for oc0 in (0, 2, 4, 6):
        tail_wout(3, oc0)

    pp.release()
    late.release()
    xtp.release()
    work.release()
    persist.release()
    wpool.release()
    consts.release()


_CACHE = {}


def _build():
    key = "nc"
    if key in _CACHE:
        return _CACHE[key]
    nc = bacc.Bacc("TRN2", target_bir_lowering=False, debug=False,
                   enable_asserts=True, num_devices=NCORES)
    io = {}
    io["xT"] = nc.dram_tensor("xT", [128, NCH, S], F16, kind="ExternalInput")
    io["trig"] = nc.dram_tensor("trig", [128, 2, S], F16, kind="ExternalInput")
    io["ident"] = nc.dram_tensor("ident", [128, 128], F16, kind="ExternalInput")
    io["tri2"] = nc.dram_tensor("tri2", [128, 2, 128], F16, kind="ExternalInput")
    io["ind8"] = nc.dram_tensor("ind8", [128, 8], F16, kind="ExternalInput")
    io["indT4"] = nc.dram_tensor("indT4", [4, 2, 128], F16, kind="ExternalInput")
    io["lnsb"] = nc.dram_tensor("lnsb", [4, 2], F32, kind="ExternalInput")
    io["wqkv"] = nc.dram_tensor("wqkv", [128, 3, NCH, 128], F16, kind="ExternalInput")
    io["wo"] = nc.dram_tensor("wo", [128, D], F16, kind="ExternalInput")
    io["v1e"] = nc.dram_tensor("v1e", [128, NKC, 130], F16, kind="ExternalInput")
    io["outp"] = nc.dram_tensor("outp", [128, 8, S], F16, kind="ExternalOutput")

    with tile.TileContext(nc) as tc:
        _emit(tc, io)
    nc.compile()
    _CACHE[key] = nc
    return nc


def _host_prep(x, v1, Wq, Wk, Wv, Wout, lambdas):
    x = np.asarray(x, np.float32).reshape(S, D)
    v1 = np.asarray(v1, np.float32).reshape(S, D)
    Wq = np.asarray(Wq, np.float32)
    Wk = np.asarray(Wk, np.float32)
    Wv = np.asarray(Wv, np.float32)
    Wout = np.asarray(Wout, np.float32)
    lam = np.float32(np.asarray(lambdas))

    # xT as [128, chunk, S] fp16
    xT = np.ascontiguousarray(
        x.T.reshape(NCH, 128, S).transpose(1, 0, 2)).astype(np.float16)

    inv_freq = (np.float32(1.0)
                / np.power(np.float32(10000.0),
                           np.arange(0, HD, 2, dtype=np.float32) / np.float32(HD)))
    tt = np.arange(S, dtype=np.float32)
    freqs = np.outer(tt, inv_freq).astype(np.float32)     # [S, 32]
    cos = np.cos(freqs).T                                  # [32, S]
    sin = np.sin(freqs).T
    cosT = np.tile(cos, (4, 1)).astype(np.float16)
    sinTs = np.concatenate([sin, -sin, sin, -sin], axis=0).astype(np.float16)
    trig = np.stack([cosT, sinTs], axis=1)    # [128, 2, S]

    ident = np.eye(128, dtype=np.float16)
    kk, qq = np.meshgrid(np.arange(128), np.arange(128), indexing="ij")
    tri01 = (qq >= kk).astype(np.float16)      # keep-mask: q-col >= k-row
    tri2 = np.ascontiguousarray(np.stack([tri01, tri01], axis=1))  # [128,2,128]

    ind8 = np.zeros((128, 8), dtype=np.float16)
    ind8[0:64, 0] = 1.0       # q h0 -> row 0
    ind8[64:128, 1] = 1.0     # q h1 -> row 1
    ind8[0:64, 4 + 2] = 1.0   # k h0 -> row 2
    ind8[64:128, 4 + 3] = 1.0  # k h1 -> row 3

    indT4 = np.zeros((4, 2, 128), dtype=np.float16)
    indT4[0, 0, 0:64] = 1.0    # q map: row0 -> dims 0..63
    indT4[1, 0, 64:128] = 1.0
    indT4[2, 1, 0:64] = 1.0    # k map: row2 -> dims 0..63
    indT4[3, 1, 64:128] = 1.0

    # inv = exp(-0.5 * ln(sum * scale + bias))
    lnsb = np.zeros((4, 2), dtype=np.float32)
    lnsb[0:2, 0] = 1.0          # q: (sum + 64eps) -> folds the 1/8 softmax scale
    lnsb[0:2, 1] = 64.0 * EPS
    lnsb[2:4, 0] = 1.0 / 64.0   # k: (mean + eps)
    lnsb[2:4, 1] = EPS

    shared = dict(xT=xT, trig=trig, ident=ident, tri2=tri2,
                  ind8=ind8, indT4=indT4, lnsb=lnsb)

    in_maps = []
    for c in range(NCORES):
        sl = slice(128 * c, 128 * c + 128)
        m = dict(shared)
        wq = Wq[sl, :].T.reshape(NCH, 128, 128).transpose(1, 0, 2)
        wk = Wk[sl, :].T.reshape(NCH, 128, 128).transpose(1, 0, 2)
        wv = (((np.float32(1.0) - lam) * Wv[sl, :]).T
              .reshape(NCH, 128, 128).transpose(1, 0, 2))
        m["wqkv"] = np.ascontiguousarray(
            np.stack([wq, wk, wv], axis=1)).astype(np.float16)
        m["wo"] = np.ascontiguousarray(Wout[:, sl].T).astype(np.float16)
        # v1e: [128 k, 16 chunk, 130] with ones at cols 64 and 129
        v1p = (lam * v1[:, sl]).reshape(NKC, 128, 2, HD)   # [chunk, k, h, hd]
        v1e = np.ones((128, NKC, 130), dtype=np.float16)
        v1e[:, :, 0:64] = v1p[:, :, 0, :].transpose(1, 0, 2).astype(np.float16)
        v1e[:, :, 65:129] = v1p[:, :, 1, :].transpose(1, 0, 2).astype(np.float16)
        m["v1e"] = v1e
        in_maps.append(m)
    return in_maps


def run(inputs, trace=False):
    nh = int(np.asarray(inputs["n_heads"]))
    assert nh == NH, f"kernel compiled for n_heads={NH}, got {nh}"
    nc = _build()
    in_maps = _host_prep(inputs["x"], inputs["v1"], inputs["Wq"], inputs["Wk"],
                         inputs["Wv"], inputs["Wout"], inputs["lambdas"])
    res = bass_utils.run_bass_kernel_spmd(
        nc, in_maps, core_ids=list(range(NCORES)), trace=trace)
    outT = np.zeros((D, S), dtype=np.float32)
    for c in range(NCORES):
        o = res.results[c]["outp"].astype(np.float32)   # [128, 8, S]
        outT += o.transpose(1, 0, 2).reshape(D, S)
    y = np.ascontiguousarray(outT.T).reshape(1, S, D).astype(np.float32)
    v1 = np.asarray(inputs["v1"], np.float32).reshape(1, S, D)
    return (y, v1), res


def kernel(**inputs):
    outs, _ = run(inputs, trace=False)
    return outs
